# revision 1
# baseline (speedup 1.0000x reference)
"""GPLinear (geometric-product linear layer) Trainium2 kernel.

Reference computation (see problem):
    out = einsum('ijk,poi,bpj->bok', GP, W, x) + b
with x (16384, 64, 16) f32, W (64, 64, 16), b (64, 16), GP (16, 16, 16)
the Cl(4,0) Cayley table (one +-1 per (i,j)).

Strategy:
  * Host: fold the tiny sparse GP into W once:
        Wg[p*16+j, o*16+k] = sum_i GP[i,j,k] * W[p,o,i]       (1024, 1024)
    so the device work is one big GEMM  out = x2 @ Wg + b  with
    x2 = x.reshape(16384, 1024).
  * Data-parallel over 8 NeuronCores: shard x2 (and out) along batch,
    2048 rows per core; replicate Wg and b.
  * Host pre-transposes each x2 shard to xT (1024, 2048) so the device
    sees the contraction dim (pj) on SBUF partitions with no on-chip
    transpose and fully contiguous DMA.
  * Device (per core): for each 128-row batch tile, accumulate
    psum(128b, 512ok) over 8 contraction chunks with fp32r matmuls
    (full-rate fp32 on the PE), add bias during the PSUM->SBUF copy on
    the vector engine, stream results back to DRAM.
"""

from contextlib import ExitStack

import numpy as np

import concourse.bass as bass
import concourse.tile as tile
from concourse import bacc, mybir
from concourse.bass import ds, ts
from concourse.bass_utils import run_bass_kernel_spmd

N_CORES = 8
P = 128
BATCH = 16384
B_PER_CORE = BATCH // N_CORES  # 2048
K_DIM = 1024  # pj = in_features * basis
N_DIM = 1024  # ok = out_features * basis
KC = K_DIM // P  # 8 contraction chunks
NH = N_DIM // 512  # 2 psum halves


def _build_nc(bt=512, xt_bufs=3, out_bufs=4, psum_bufs=4):
    mm_dtype = mybir.dt.float32r
    nc = bacc.Bacc("TRN2", target_bir_lowering=False, debug=False,
                   num_devices=N_CORES)

    xt_d = nc.dram_tensor("xt", [K_DIM, B_PER_CORE], mm_dtype,
                          kind="ExternalInput").ap()
    wg_d = nc.dram_tensor("wg", [K_DIM, N_DIM], mm_dtype,
                          kind="ExternalInput").ap()
    bias_d = nc.dram_tensor("bias", [P, N_DIM], mybir.dt.float32,
                            kind="ExternalInput").ap()
    out_d = nc.dram_tensor("out", [B_PER_CORE, N_DIM], mybir.dt.float32,
                           kind="ExternalOutput").ap()

    n_bt = B_PER_CORE // bt
    n_sub = bt // P

    with tile.TileContext(nc) as tc:
        with ExitStack() as ctx:
            wg_pool = ctx.enter_context(tc.tile_pool(name="wg", bufs=1))
            const_pool = ctx.enter_context(tc.tile_pool(name="const", bufs=1))
            xt_pool = ctx.enter_context(tc.tile_pool(name="xt", bufs=xt_bufs))
            out_pool = ctx.enter_context(tc.tile_pool(name="out", bufs=out_bufs))
            psum_pool = ctx.enter_context(
                tc.tile_pool(name="psum", bufs=psum_bufs, space="PSUM"))

            wg_sb = []
            for kc in range(KC):
                t = wg_pool.tile([P, N_DIM], mm_dtype, tag=f"wg{kc}")
                nc.sync.dma_start(t[:], wg_d[ts(kc, P), :])
                wg_sb.append(t)

            bias_sb = const_pool.tile([P, N_DIM], mybir.dt.float32)
            nc.sync.dma_start(bias_sb[:], bias_d[:])

            for bti in range(n_bt):
                xt_t = xt_pool.tile([P, KC, bt], mm_dtype, tag="xt")
                nc.sync.dma_start(
                    xt_t[:],
                    xt_d.rearrange("(kc p) b -> p kc b", p=P)[:, :, ts(bti, bt)],
                )
                for sub in range(n_sub):
                    brow = bti * bt + sub * P
                    out_t = out_pool.tile([P, N_DIM], mybir.dt.float32, tag="out")
                    for oh in range(NH):
                        ps = psum_pool.tile([P, 512], mybir.dt.float32, tag="ps")
                        for kc in range(KC):
                            nc.tensor.matmul(
                                ps[:],
                                lhsT=xt_t[:, kc, ts(sub, P)],
                                rhs=wg_sb[kc][:, ts(oh, 512)],
                                start=(kc == 0),
                                stop=(kc == KC - 1),
                            )
                        nc.vector.tensor_add(
                            out=out_t[:, ts(oh, 512)],
                            in0=ps[:],
                            in1=bias_sb[:, ts(oh, 512)],
                        )
                    nc.sync.dma_start(out_d[ds(brow, P), :], out_t[:])

    nc.compile()
    return nc


_NC_CACHE = {}


def _get_nc():
    if "nc" not in _NC_CACHE:
        _NC_CACHE["nc"] = _build_nc()
    return _NC_CACHE["nc"]


def kernel(x, W, b, GP):
    x = np.asarray(x, dtype=np.float32)
    W = np.asarray(W, dtype=np.float32)
    b = np.asarray(b, dtype=np.float32)
    GP = np.asarray(GP, dtype=np.float32)

    # Fold the Cayley table into the weights (tiny):
    # Wg[p, j, o, k] = sum_i GP[i, j, k] * W[p, o, i]
    Wg = np.einsum("ijk,poi->pjok", GP, W).reshape(K_DIM, N_DIM)
    Wg = np.ascontiguousarray(Wg, dtype=np.float32)
    bias = np.ascontiguousarray(
        np.broadcast_to(b.reshape(1, N_DIM), (P, N_DIM)), dtype=np.float32)

    x2 = x.reshape(BATCH, K_DIM)
    in_maps = []
    for c in range(N_CORES):
        xt = np.ascontiguousarray(x2[c * B_PER_CORE:(c + 1) * B_PER_CORE, :].T)
        in_maps.append({"xt": xt, "wg": Wg, "bias": bias})

    nc = _get_nc()
    res = run_bass_kernel_spmd(nc, in_maps, list(range(N_CORES)))
    out = np.concatenate(
        [np.asarray(res.results[c]["out"]) for c in range(N_CORES)], axis=0)
    return np.ascontiguousarray(out.reshape(BATCH, 64, 16), dtype=np.float32)
